# revision 12
# baseline (speedup 1.0000x reference)
"""Trainium2 Bass kernel for AttLocRec (location-aware recurrent attention).

Self-contained: takes FULL inputs (as produced by setup_inputs()), shards
batch across 8 NeuronCores, runs one SPMD Bass kernel, gathers outputs.

Math (per batch row b):
  pre_enc = enc @ W_enc.T + b_enc                  (T, A)
  att_feat = maxpool_T(relu(conv1d(att_prev)))     (C,)
  gates = att_feat @ W_ih.T + att_h @ W_hh.T       (4A,)
  c' = sig(f)*att_c + sig(i)*tanh(g); h' = sig(o)*tanh(c')
  e = tanh(pre_enc + h' + dec_z @ W_dec.T) @ W_g   (T,)   [+b_g, cancels in softmax]
  w = softmax(2*e + mask)                          (T,)
  ctx = w @ enc                                    (E,)

Device layout: scores computed transposed (A on partitions, T on free) so the
(h'+dec_proj+b_enc) bias is a per-partition ACT bias fused into tanh;
e-reduction over A and ctx-reduction over T are PE matmuls. Score matmuls run
as float32r (full-rate fp32); ctx pass streams a bf16 copy of enc.
"""

import sys

for _p in ("/opt/trn_rl_repo", "/root/.axon_site/_ro/trn_rl_repo"):
    if _p not in sys.path:
        sys.path.insert(0, _p)

import numpy as np
import ml_dtypes

B, T, E, D, A, C, FILT = 32, 1600, 1024, 1024, 512, 10, 100
KF = 2 * FILT + 1          # 201 conv taps
G4 = 4 * A                 # 2048 stacked gates
D1 = D + 1                 # dec_z with ones column (folds b_enc)
NCORES = 8
NB = B // NCORES           # 4 batch rows per core
NEG = -2.0e30              # additive mask in scaled-score units
TCH = [(0, 512), (512, 512), (1024, 512), (1536, 64)]   # T chunks (N<=512)
NTK = 13                   # ceil(T/128) K-chunks over T for ctx

_BUILT = None


def _build():
    import concourse.bacc as bacc
    import concourse.tile as tile
    import concourse.mybir as mybir

    f32 = mybir.dt.float32
    f32r = mybir.dt.float32r
    bf16 = mybir.dt.bfloat16
    AF = mybir.ActivationFunctionType
    ALU = mybir.AluOpType
    AX = mybir.AxisListType

    nc = bacc.Bacc("TRN2", target_bir_lowering=False, debug=False,
                   num_devices=NCORES)

    def din(name, shape, dt=f32):
        return nc.dram_tensor(name, list(shape), dt, kind="ExternalInput").ap()

    def dout(name, shape, dt=f32):
        return nc.dram_tensor(name, list(shape), dt, kind="ExternalOutput").ap()

    encT = din("encT", (NB, E, T), f32r)      # enc transposed (fp32r-rounded)
    encN = din("encN", (NB, T, E), bf16)      # enc natural, bf16 (ctx pass)
    wencT = din("wencT", (E, A), f32r)
    wg = din("wg", (A, 1), f32r)
    xcv = din("xcv", (NB, KF, T), f32r)       # im2col'd att_prev (padded)
    cvT = din("cvT", (KF, C), f32r)
    wihT = din("wihT", (C, G4), f32r)
    whhT = din("whhT", (A, G4), f32r)
    athT = din("athT", (A, NB), f32r)
    atc = din("atc", (NB, A))
    dzT = din("dzT", (D1, NB), f32r)          # [dec_z.T; ones]
    wdT = din("wdT", (D1, A), f32r)           # [W_dec.T; b_enc]
    maskS = din("maskS", (NB, T))             # 0 valid / NEG padded
    ident = din("ident", (128, 128))

    octx = dout("octx", (NB, E))
    ow = dout("ow", (NB, T))
    oh = dout("oh", (NB, A))
    oc = dout("oc", (NB, A))

    r = lambda ap: ap

    with tile.TileContext(nc) as tc:
        with tc.tile_pool(name="persist", bufs=1) as pp, \
             tc.tile_pool(name="small", bufs=1) as ps, \
             tc.tile_pool(name="encT", bufs=2) as pT:

            ident_sb = pp.tile([128, 128], f32)
            nc.sync.dma_start(ident_sb[:], ident[:])
            wencT_sb = pp.tile([128, 8, A], f32r)
            nc.sync.dma_start(wencT_sb[:],
                              wencT.rearrange("(k p) a -> p k a", p=128))
            wg_sb = pp.tile([128, 4, 1], f32r)
            nc.sync.dma_start(wg_sb[:],
                              wg.rearrange("(k p) o -> p k o", p=128))
            atc_sb = ps.tile([NB, A], f32)
            nc.sync.dma_start(atc_sb[:], atc[:])

            featT = ps.tile([C, NB], f32r)
            cmax = ps.tile([C, NB], f32)

            # ---------------- phase 0a: location conv ----------------
            with tc.tile_pool(name="xp", bufs=2) as px, \
                 tc.tile_pool(name="cvp", bufs=1) as pcv, \
                 tc.tile_pool(name="cps", bufs=1, space="PSUM") as pcp:
                cvT0 = pcv.tile([128, C], f32r)
                nc.sync.dma_start(cvT0[:], cvT[0:128, :])
                cvT1 = pcv.tile([KF - 128, C], f32r)
                nc.sync.dma_start(cvT1[:], cvT[128:KF, :])
                for b in range(NB):
                    x0 = px.tile([128, T], f32r, tag="x0")
                    nc.sync.dma_start(x0[:], xcv[b, 0:128, :])
                    x1 = px.tile([KF - 128, T], f32r, tag="x1")
                    nc.sync.dma_start(x1[:], xcv[b, 128:KF, :])
                    cps = pcp.tile([C, T], f32)
                    for (t0, tw) in TCH:
                        nc.tensor.matmul(cps[:, t0:t0 + tw], r(cvT0[:]),
                                         r(x0[:, t0:t0 + tw]),
                                         start=True, stop=False)
                        nc.tensor.matmul(cps[:, t0:t0 + tw], r(cvT1[:]),
                                         r(x1[:, t0:t0 + tw]),
                                         start=False, stop=True)
                    nc.vector.tensor_reduce(cmax[:, b:b + 1], cps[:],
                                            axis=AX.X, op=ALU.max)
                nc.scalar.activation(featT[:], cmax[:], AF.Relu)

            # ---------------- phase 0b: LSTM cell + dec proj ----------------
            with tc.tile_pool(name="w0", bufs=2) as pw0, \
                 tc.tile_pool(name="gps", bufs=1, space="PSUM") as pg, \
                 tc.tile_pool(name="btp", bufs=1, space="PSUM") as pbt:
                wihT_sb = pw0.tile([C, G4], f32r)
                nc.sync.dma_start(wihT_sb[:], wihT[:])
                athT_sb = pw0.tile([128, 4, NB], f32r)
                nc.sync.dma_start(athT_sb[:],
                                  athT.rearrange("(k p) b -> p k b", p=128))
                dzT_sb = pw0.tile([128, 8, NB], f32r)
                nc.sync.dma_start(dzT_sb[:],
                                  dzT[0:D, :].rearrange("(k p) b -> p k b", p=128))
                dzT_tl = pw0.tile([1, NB], f32r)
                nc.sync.dma_start(dzT_tl[:], dzT[D:D1, :])
                wdT_sb = pw0.tile([128, 8, A], f32r)
                nc.sync.dma_start(wdT_sb[:],
                                  wdT[0:D, :].rearrange("(k p) a -> p k a", p=128))
                wdT_tl = pw0.tile([1, A], f32r)
                nc.sync.dma_start(wdT_tl[:], wdT[D:D1, :])

                gps = pg.tile([NB, G4], f32)
                for ci in range(4):
                    c0 = ci * 512
                    whh_c = pw0.tile([128, 4, 512], f32r, tag="whhc")
                    nc.sync.dma_start(
                        whh_c[:], whhT[:, c0:c0 + 512].rearrange(
                            "(k p) g -> p k g", p=128))
                    nc.tensor.matmul(gps[:, c0:c0 + 512], r(featT[:]),
                                     r(wihT_sb[:, c0:c0 + 512]),
                                     start=True, stop=False)
                    for k in range(4):
                        nc.tensor.matmul(gps[:, c0:c0 + 512],
                                         r(athT_sb[:, k, :]),
                                         r(whh_c[:, k, :]),
                                         start=False, stop=(k == 3))
                dps = pg.tile([NB, A], f32)
                for k in range(8):
                    nc.tensor.matmul(dps[:], r(dzT_sb[:, k, :]),
                                     r(wdT_sb[:, k, :]),
                                     start=(k == 0), stop=False)
                nc.tensor.matmul(dps[:], r(dzT_tl[:]), r(wdT_tl[:]),
                                 start=False, stop=True)

                sigi = ps.tile([NB, A], f32)
                nc.scalar.activation(sigi[:], gps[:, 0:A], AF.Sigmoid)
                sigf = ps.tile([NB, A], f32)
                nc.scalar.activation(sigf[:], gps[:, A:2 * A], AF.Sigmoid)
                tag = ps.tile([NB, A], f32)
                nc.scalar.activation(tag[:], gps[:, 2 * A:3 * A], AF.Tanh)
                sigo = ps.tile([NB, A], f32)
                nc.scalar.activation(sigo[:], gps[:, 3 * A:4 * A], AF.Sigmoid)

                t1 = ps.tile([NB, A], f32)
                nc.vector.tensor_mul(t1[:], sigf[:], atc_sb[:])
                t2 = ps.tile([NB, A], f32)
                nc.vector.tensor_mul(t2[:], sigi[:], tag[:])
                cn = ps.tile([NB, A], f32)
                nc.vector.tensor_add(cn[:], t1[:], t2[:])
                tac = ps.tile([NB, A], f32)
                nc.scalar.activation(tac[:], cn[:], AF.Tanh)
                hn = ps.tile([NB, A], f32)
                nc.vector.tensor_mul(hn[:], sigo[:], tac[:])
                nc.sync.dma_start(oc[:], cn[:])
                nc.sync.dma_start(oh[:], hn[:])

                bias_nat = ps.tile([NB, A], f32)
                nc.vector.tensor_add(bias_nat[:], hn[:], dps[:])
                biasT = []
                for a in range(4):
                    tr = pbt.tile([128, NB], f32)
                    nc.tensor.transpose(tr[:], bias_nat[:, a * 128:(a + 1) * 128],
                                        ident_sb[0:NB, 0:NB])
                    bt = pp.tile([128, NB], f32, tag=f"biasT{a}")
                    nc.scalar.activation(bt[:], tr[:], AF.Copy)
                    biasT.append(bt)

            # ---------------- phase 1: scores + softmax + ctx, per batch ----
            with tc.tile_pool(name="encN", bufs=1) as pN, \
                 tc.tile_pool(name="tanhp", bufs=6) as ptn, \
                 tc.tile_pool(name="wTp", bufs=26) as pwT, \
                 tc.tile_pool(name="sm", bufs=2) as psm, \
                 tc.tile_pool(name="psP", bufs=2, space="PSUM") as pP, \
                 tc.tile_pool(name="psE", bufs=2, space="PSUM") as pE, \
                 tc.tile_pool(name="psW", bufs=2, space="PSUM") as pW, \
                 tc.tile_pool(name="psC", bufs=1, space="PSUM") as pC:

                for b in range(NB):
                    et0 = pT.tile([128, 4, T], f32r, tag="et")
                    et1 = pT.tile([128, 4, T], f32r, tag="et")
                    for j in range(2):
                        nc.sync.dma_start(
                            et0[:, 2 * j:2 * j + 2, :],
                            encT[b, 256 * j:256 * (j + 1), :].rearrange(
                                "(k p) t -> p k t", p=128))
                        nc.sync.dma_start(
                            et1[:, 2 * j:2 * j + 2, :],
                            encT[b, 512 + 256 * j:512 + 256 * (j + 1), :]
                            .rearrange("(k p) t -> p k t", p=128))
                    ehalf = (et0, et1)
                    en = pN.tile([128, 12, E], bf16, tag="en")
                    for j in range(4):
                        nc.sync.dma_start(
                            en[:, 3 * j:3 * j + 3, :],
                            encN[b, 384 * j:384 * (j + 1), :].rearrange(
                                "(k p) e -> p k e", p=128))
                    ent = pN.tile([64, E], bf16, tag="ent")
                    nc.sync.dma_start(ent[:], encN[b, 1536:T, :])

                    mrow = psm.tile([1, T], f32, tag="mask")
                    nc.sync.dma_start(mrow[:], maskS[b:b + 1, :])
                    e_b = psm.tile([1, T], f32, tag="e")
                    m_b = psm.tile([1, 1], f32, tag="m")
                    nm_b = psm.tile([1, 1], f32, tag="nm")
                    s_b = psm.tile([1, 1], f32, tag="s")
                    rs_b = psm.tile([1, 1], f32, tag="rs")

                    # ---- scores ----
                    for (t0, tw) in TCH:
                        tts = []
                        for a in range(4):
                            pps = pP.tile([128, 512], f32)
                            for k in range(8):
                                nc.tensor.matmul(
                                    pps[:, 0:tw],
                                    r(wencT_sb[:, k, a * 128:(a + 1) * 128]),
                                    r(ehalf[k // 4][:, k % 4, t0:t0 + tw]),
                                    start=(k == 0), stop=(k == 7))
                            th = ptn.tile([128, 512], f32r)
                            nc.scalar.activation(th[:, 0:tw], pps[:, 0:tw],
                                                 AF.Tanh,
                                                 bias=biasT[a][:, b:b + 1])
                            tts.append(th)
                        pe_ = pE.tile([1, 512], f32)
                        for a in range(4):
                            nc.tensor.matmul(pe_[:, 0:tw], r(wg_sb[:, a, :]),
                                             r(tts[a][:, 0:tw]),
                                             start=(a == 0), stop=(a == 3))
                        nc.vector.scalar_tensor_tensor(
                            e_b[:, t0:t0 + tw], pe_[:, 0:tw], 2.0,
                            mrow[:, t0:t0 + tw], op0=ALU.mult, op1=ALU.add)

                    # ---- masked softmax (all on partition 0) ----
                    nc.vector.tensor_reduce(m_b[:], e_b[:],
                                            axis=AX.X, op=ALU.max)
                    nc.vector.tensor_scalar_mul(nm_b[:], m_b[:], -1.0)
                    nc.scalar.activation(e_b[:], e_b[:], AF.Exp,
                                         bias=nm_b[:], scale=1.0,
                                         accum_out=s_b[:])
                    nc.vector.reciprocal(rs_b[:], s_b[:])
                    nc.vector.tensor_scalar_mul(e_b[:], e_b[:], rs_b[:])
                    nc.sync.dma_start(ow[b:b + 1, :], e_b[:])

                    # ---- w transpose + ctx ----
                    pc0 = pC.tile([1, 512], f32, tag="pc0")
                    pc1 = pC.tile([1, 512], f32, tag="pc1")
                    for k in range(NTK):
                        kw = 128 if k < 12 else 64
                        trp = pW.tile([128, 1], f32)
                        nc.tensor.transpose(trp[0:kw, :],
                                            e_b[:, k * 128:k * 128 + kw],
                                            ident_sb[0:1, 0:1])
                        wtk = pwT.tile([128, 1], bf16)
                        nc.vector.tensor_copy(wtk[0:kw, :], trp[0:kw, :])
                        rhs = en[:, k, :] if k < 12 else ent[:]
                        nc.tensor.matmul(pc0[:], wtk[0:kw, :],
                                         rhs[0:kw, 0:512],
                                         start=(k == 0), stop=(k == NTK - 1))
                        nc.tensor.matmul(pc1[:], wtk[0:kw, :],
                                         rhs[0:kw, 512:1024],
                                         start=(k == 0), stop=(k == NTK - 1))
                    ctx_b = psm.tile([1, E], f32, tag="ctx")
                    nc.scalar.activation(ctx_b[:, 0:512], pc0[:], AF.Copy)
                    nc.scalar.activation(ctx_b[:, 512:1024], pc1[:], AF.Copy)
                    nc.sync.dma_start(octx[b:b + 1, :], ctx_b[:])

    nc.compile()
    return nc


def _round_f32r(x):
    """Round fp32 to fp32r (11-bit mantissa, RNE) — what the PE consumes."""
    u = np.ascontiguousarray(x, np.float32).view(np.uint32)
    lsb = (u >> np.uint32(12)) & np.uint32(1)
    u = u + np.uint32(0x7FF) + lsb
    u &= np.uint32(0xFFFFF000)
    return u.view(np.float32)


def _prep_inputs(enc_hs_pad, enc_hs_len, dec_z, att_prev, att_h, att_c,
                 W_enc, b_enc, W_dec, conv_w, W_ih, W_hh, W_g, b_g):
    bf = ml_dtypes.bfloat16
    f = np.float32

    encT32 = _round_f32r(np.ascontiguousarray(enc_hs_pad.transpose(0, 2, 1),
                                               dtype=f))
    encN32 = np.ascontiguousarray(enc_hs_pad, dtype=bf)

    att_pad = np.zeros((B, T + 2 * FILT), dtype=f)
    att_pad[:, FILT:FILT + T] = att_prev
    xcv = np.ascontiguousarray(
        np.lib.stride_tricks.sliding_window_view(att_pad, T, axis=1)
        .transpose(0, 1, 2))  # (B, KF, T)

    wencT = _round_f32r(np.ascontiguousarray(W_enc.T, dtype=f))
    wg = _round_f32r(np.ascontiguousarray(W_g[:, None], dtype=f))
    cvT = _round_f32r(np.ascontiguousarray(conv_w[:, 0, :].T, dtype=f))
    wihT = _round_f32r(np.ascontiguousarray(W_ih.T, dtype=f))
    whhT = _round_f32r(np.ascontiguousarray(W_hh.T, dtype=f))
    wdT = _round_f32r(np.ascontiguousarray(
        np.concatenate([W_dec.T, b_enc[None, :]], axis=0), dtype=f))
    maskS = np.where(np.arange(T)[None, :] < np.asarray(enc_hs_len)[:, None],
                     np.float32(0.0), np.float32(NEG)).astype(f)
    ident = np.eye(128, dtype=f)

    dzT_full = _round_f32r(np.concatenate(
        [np.asarray(dec_z, dtype=f).T, np.ones((1, B), dtype=f)], axis=0))
    athT_full = _round_f32r(np.ascontiguousarray(np.asarray(att_h, dtype=f).T))

    in_maps = []
    for c in range(NCORES):
        s = slice(c * NB, (c + 1) * NB)
        in_maps.append({
            "encT": np.ascontiguousarray(encT32[s]),
            "encN": np.ascontiguousarray(encN32[s]),
            "wencT": wencT,
            "wg": wg,
            "xcv": _round_f32r(np.ascontiguousarray(xcv[s])),
            "cvT": cvT,
            "wihT": wihT,
            "whhT": whhT,
            "athT": np.ascontiguousarray(athT_full[:, s]),
            "atc": np.ascontiguousarray(np.asarray(att_c, dtype=f)[s]),
            "dzT": np.ascontiguousarray(dzT_full[:, s]),
            "wdT": wdT,
            "maskS": np.ascontiguousarray(maskS[s]),
            "ident": ident,
        })
    return in_maps


def kernel(**inputs):
    global _BUILT
    from concourse import bass_utils

    if _BUILT is None:
        _BUILT = _build()
    nc = _BUILT

    in_maps = _prep_inputs(**{k: np.asarray(v) for k, v in inputs.items()})
    res = bass_utils.run_bass_kernel_spmd(nc, in_maps,
                                          core_ids=list(range(NCORES)))
    ctx = np.concatenate([res.results[c]["octx"] for c in range(NCORES)], 0)
    w = np.concatenate([res.results[c]["ow"] for c in range(NCORES)], 0)
    h = np.concatenate([res.results[c]["oh"] for c in range(NCORES)], 0)
    cc = np.concatenate([res.results[c]["oc"] for c in range(NCORES)], 0)
    return ctx, w, h, cc


# revision 15
# speedup vs baseline: 1.0402x; 1.0402x over previous
"""Trainium2 Bass kernel for AttLocRec (location-aware recurrent attention).

Self-contained: takes FULL inputs (as produced by setup_inputs()), shards
batch across 8 NeuronCores, runs one SPMD Bass kernel, gathers outputs.

Math (per batch row b):
  pre_enc = enc @ W_enc.T + b_enc                  (T, A)
  att_feat = maxpool_T(relu(conv1d(att_prev)))     (C,)
  gates = att_feat @ W_ih.T + att_h @ W_hh.T       (4A,)
  c' = sig(f)*att_c + sig(i)*tanh(g); h' = sig(o)*tanh(c')
  e = tanh(pre_enc + h' + dec_z @ W_dec.T) @ W_g   (T,)   [+b_g, cancels in softmax]
  w = softmax(2*e + mask)                          (T,)
  ctx = w @ enc                                    (E,)

Device layout: scores computed transposed (A on partitions, T on free) so the
(h'+dec_proj+b_enc) bias is a per-partition ACT bias fused into tanh;
e-reduction over A and ctx-reduction over T are PE matmuls. Score matmuls run
as float32r (full-rate fp32); ctx pass streams a bf16 copy of enc.
"""

import sys

for _p in ("/opt/trn_rl_repo", "/root/.axon_site/_ro/trn_rl_repo"):
    if _p not in sys.path:
        sys.path.insert(0, _p)

import numpy as np
import ml_dtypes

B, T, E, D, A, C, FILT = 32, 1600, 1024, 1024, 512, 10, 100
KF = 2 * FILT + 1          # 201 conv taps
G4 = 4 * A                 # 2048 stacked gates
D1 = D + 1                 # dec_z with ones column (folds b_enc)
NCORES = 8
NB = B // NCORES           # 4 batch rows per core
NEG = -2.0e30              # additive mask in scaled-score units
TCH = [(0, 512), (512, 512), (1024, 512), (1536, 64)]   # T chunks (N<=512)
NTK = 13                   # ceil(T/128) K-chunks over T for ctx

_BUILT = None


def _build(reps=1):
    import concourse.bacc as bacc
    import concourse.tile as tile
    import concourse.mybir as mybir

    f32 = mybir.dt.float32
    f32r = mybir.dt.float32r
    bf16 = mybir.dt.bfloat16
    AF = mybir.ActivationFunctionType
    ALU = mybir.AluOpType
    AX = mybir.AxisListType

    nc = bacc.Bacc("TRN2", target_bir_lowering=False, debug=False,
                   num_devices=NCORES)

    def din(name, shape, dt=f32):
        return nc.dram_tensor(name, list(shape), dt, kind="ExternalInput").ap()

    def dout(name, shape, dt=f32):
        return nc.dram_tensor(name, list(shape), dt, kind="ExternalOutput").ap()

    # all tensors pre-swizzled on host: every DMA is contiguous per partition
    encT = din("encT", (NB, 2, 128, 4, T), f32r)   # [b, half, p, k, t]
    encN = din("encN", (NB, 128, 12, E), bf16)     # [b, p, k, e] (t=k*128+p)
    encNt = din("encNt", (NB, 64, E), bf16)        # t tail 1536..1600
    wencT = din("wencT", (128, 8, A), f32r)
    wg = din("wg", (128, 4, 1), f32r)
    xcv = din("xcv", (NB, KF, T), f32r)            # im2col'd att_prev (padded)
    cvT = din("cvT", (KF, C), f32r)
    wihT = din("wihT", (C, G4), f32r)
    whh = din("whh", (4, 128, 4, 512), bf16)       # [ci, p, k, j]
    athT = din("athT", (128, 4, NB), bf16)
    atc = din("atc", (NB, A))
    dzT = din("dzT", (128, 8, NB), f32r)           # dec_z.T swizzled
    dzTt = din("dzTt", (1, NB), f32r)              # ones row
    wdT = din("wdT", (128, 8, A), f32r)            # W_dec.T swizzled
    wdTt = din("wdTt", (1, A), f32r)               # b_enc row
    maskS = din("maskS", (NB, T))             # 0 valid / NEG padded
    ident = din("ident", (128, 128))

    octx = dout("octx", (NB, E))
    ow = dout("ow", (NB, T))
    oh = dout("oh", (NB, A))
    oc = dout("oc", (NB, A))

    r = lambda ap: ap

    with tile.TileContext(nc) as tc:
      for _rep in range(reps):
        with tc.tile_pool(name="persist", bufs=1) as pp, \
             tc.tile_pool(name="small", bufs=1) as ps, \
             tc.tile_pool(name="encT", bufs=2) as pT:

            ident_sb = pp.tile([128, 128], f32)
            nc.sync.dma_start(ident_sb[:], ident[:])
            wencT_sb = pp.tile([128, 8, A], f32r)
            nc.sync.dma_start(wencT_sb[:], wencT[:])
            wg_sb = pp.tile([128, 4, 1], f32r)
            nc.sync.dma_start(wg_sb[:], wg[:])

            # ---------------- phase 0a/0b shared small pool ----------------
            ph0 = tc.tile_pool(name="ph0", bufs=1)
            ps0 = ph0.__enter__()
            atc_sb = ps0.tile([NB, A], f32)
            nc.sync.dma_start(atc_sb[:], atc[:])
            featT = ps0.tile([C, NB], f32r)
            cmax = ps0.tile([C, NB], f32)

            # ---------------- phase 0a: location conv ----------------
            with tc.tile_pool(name="xp", bufs=2) as px, \
                 tc.tile_pool(name="cvp", bufs=1) as pcv, \
                 tc.tile_pool(name="cps", bufs=1, space="PSUM") as pcp:
                cvT0 = pcv.tile([128, C], f32r)
                nc.sync.dma_start(cvT0[:], cvT[0:128, :])
                cvT1 = pcv.tile([KF - 128, C], f32r)
                nc.sync.dma_start(cvT1[:], cvT[128:KF, :])
                for b in range(NB):
                    x0 = px.tile([128, T], f32r, tag="x0")
                    nc.sync.dma_start(x0[:], xcv[b, 0:128, :])
                    x1 = px.tile([KF - 128, T], f32r, tag="x1")
                    nc.sync.dma_start(x1[:], xcv[b, 128:KF, :])
                    cps = pcp.tile([C, T], f32)
                    for (t0, tw) in TCH:
                        nc.tensor.matmul(cps[:, t0:t0 + tw], r(cvT0[:]),
                                         r(x0[:, t0:t0 + tw]),
                                         start=True, stop=False)
                        nc.tensor.matmul(cps[:, t0:t0 + tw], r(cvT1[:]),
                                         r(x1[:, t0:t0 + tw]),
                                         start=False, stop=True)
                    nc.vector.tensor_reduce(cmax[:, b:b + 1], cps[:],
                                            axis=AX.X, op=ALU.max)
                nc.scalar.activation(featT[:], cmax[:], AF.Relu)

            # ---------------- phase 0b: LSTM cell + dec proj ----------------
            with tc.tile_pool(name="w0", bufs=2) as pw0, \
                 tc.tile_pool(name="gps", bufs=1, space="PSUM") as pg, \
                 tc.tile_pool(name="btp", bufs=1, space="PSUM") as pbt:
                wihT_sb = pw0.tile([C, G4], f32r)
                nc.sync.dma_start(wihT_sb[:], wihT[:])
                athT_sb = pw0.tile([128, 4, NB], bf16)
                nc.sync.dma_start(athT_sb[:], athT[:])
                dzT_sb = pw0.tile([128, 8, NB], f32r)
                nc.sync.dma_start(dzT_sb[:], dzT[:])
                dzT_tl = pw0.tile([1, NB], f32r)
                nc.sync.dma_start(dzT_tl[:], dzTt[:])
                wdT_sb = pw0.tile([128, 8, A], f32r)
                nc.sync.dma_start(wdT_sb[:], wdT[:])
                wdT_tl = pw0.tile([1, A], f32r)
                nc.sync.dma_start(wdT_tl[:], wdTt[:])

                gps = pg.tile([NB, G4], f32)
                for ci in range(4):
                    c0 = ci * 512
                    whh_c = pw0.tile([128, 4, 512], bf16, tag="whhc")
                    nc.sync.dma_start(whh_c[:], whh[ci])
                    nc.tensor.matmul(gps[:, c0:c0 + 512], r(featT[:]),
                                     r(wihT_sb[:, c0:c0 + 512]),
                                     start=True, stop=False)
                    for k in range(4):
                        nc.tensor.matmul(gps[:, c0:c0 + 512],
                                         athT_sb[:, k, :],
                                         whh_c[:, k, :],
                                         start=False, stop=(k == 3))
                dps = pg.tile([NB, A], f32)
                for k in range(8):
                    nc.tensor.matmul(dps[:], r(dzT_sb[:, k, :]),
                                     r(wdT_sb[:, k, :]),
                                     start=(k == 0), stop=False)
                nc.tensor.matmul(dps[:], r(dzT_tl[:]), r(wdT_tl[:]),
                                 start=False, stop=True)

                sigi = ps0.tile([NB, A], f32)
                nc.scalar.activation(sigi[:], gps[:, 0:A], AF.Sigmoid)
                sigf = ps0.tile([NB, A], f32)
                nc.scalar.activation(sigf[:], gps[:, A:2 * A], AF.Sigmoid)
                tag = ps0.tile([NB, A], f32)
                nc.scalar.activation(tag[:], gps[:, 2 * A:3 * A], AF.Tanh)
                sigo = ps0.tile([NB, A], f32)
                nc.scalar.activation(sigo[:], gps[:, 3 * A:4 * A], AF.Sigmoid)

                t1 = ps0.tile([NB, A], f32)
                nc.vector.tensor_mul(t1[:], sigf[:], atc_sb[:])
                t2 = ps0.tile([NB, A], f32)
                nc.vector.tensor_mul(t2[:], sigi[:], tag[:])
                cn = ps0.tile([NB, A], f32)
                nc.vector.tensor_add(cn[:], t1[:], t2[:])
                tac = ps0.tile([NB, A], f32)
                nc.scalar.activation(tac[:], cn[:], AF.Tanh)
                hn = ps0.tile([NB, A], f32)
                nc.vector.tensor_mul(hn[:], sigo[:], tac[:])
                nc.sync.dma_start(oc[:], cn[:])
                nc.sync.dma_start(oh[:], hn[:])

                bias_nat = ps0.tile([NB, A], f32)
                nc.vector.tensor_add(bias_nat[:], hn[:], dps[:])
                biasT = []
                for a in range(4):
                    tr = pbt.tile([128, NB], f32)
                    nc.tensor.transpose(tr[:], bias_nat[:, a * 128:(a + 1) * 128],
                                        ident_sb[0:NB, 0:NB])
                    bt = pp.tile([128, NB], f32, tag=f"biasT{a}")
                    nc.scalar.activation(bt[:], tr[:], AF.Copy)
                    biasT.append(bt)

            ph0.__exit__(None, None, None)

            # ---------------- phase 1: scores + softmax + ctx, per batch ----
            with tc.tile_pool(name="encN", bufs=2) as pN, \
                 tc.tile_pool(name="tanhp", bufs=6) as ptn, \
                 tc.tile_pool(name="wTp", bufs=26) as pwT, \
                 tc.tile_pool(name="sm", bufs=2) as psm, \
                 tc.tile_pool(name="psP", bufs=2, space="PSUM") as pP, \
                 tc.tile_pool(name="psE", bufs=2, space="PSUM") as pE, \
                 tc.tile_pool(name="psW", bufs=2, space="PSUM") as pW, \
                 tc.tile_pool(name="psC", bufs=1, space="PSUM") as pC:

                for b in range(NB):
                    et0 = pT.tile([128, 4, T], f32r, tag="et")
                    et1 = pT.tile([128, 4, T], f32r, tag="et")
                    nc.sync.dma_start(et0[:], encT[b, 0])
                    nc.sync.dma_start(et1[:], encT[b, 1])
                    ehalf = (et0, et1)
                    en = pN.tile([128, 12, E], bf16, tag="en")
                    nc.sync.dma_start(en[:], encN[b])
                    ent = pN.tile([64, E], bf16, tag="ent")
                    nc.sync.dma_start(ent[:], encNt[b])

                    mrow = psm.tile([1, T], f32, tag="mask")
                    nc.sync.dma_start(mrow[:], maskS[b:b + 1, :])
                    e_b = psm.tile([1, T], f32, tag="e")
                    m_b = psm.tile([1, 1], f32, tag="m")
                    nm_b = psm.tile([1, 1], f32, tag="nm")
                    s_b = psm.tile([1, 1], f32, tag="s")
                    rs_b = psm.tile([1, 1], f32, tag="rs")

                    # ---- scores ----
                    for (t0, tw) in TCH:
                        tts = []
                        for a in range(4):
                            pps = pP.tile([128, 512], f32)
                            for k in range(8):
                                nc.tensor.matmul(
                                    pps[:, 0:tw],
                                    r(wencT_sb[:, k, a * 128:(a + 1) * 128]),
                                    r(ehalf[k // 4][:, k % 4, t0:t0 + tw]),
                                    start=(k == 0), stop=(k == 7))
                            th = ptn.tile([128, 512], f32r)
                            nc.scalar.activation(th[:, 0:tw], pps[:, 0:tw],
                                                 AF.Tanh,
                                                 bias=biasT[a][:, b:b + 1])
                            tts.append(th)
                        pe_ = pE.tile([1, 512], f32)
                        for a in range(4):
                            nc.tensor.matmul(pe_[:, 0:tw], r(wg_sb[:, a, :]),
                                             r(tts[a][:, 0:tw]),
                                             start=(a == 0), stop=(a == 3))
                        nc.vector.scalar_tensor_tensor(
                            e_b[:, t0:t0 + tw], pe_[:, 0:tw], 2.0,
                            mrow[:, t0:t0 + tw], op0=ALU.mult, op1=ALU.add)

                    # ---- masked softmax (all on partition 0) ----
                    nc.vector.tensor_reduce(m_b[:], e_b[:],
                                            axis=AX.X, op=ALU.max)
                    nc.vector.tensor_scalar_mul(nm_b[:], m_b[:], -1.0)
                    nc.scalar.activation(e_b[:], e_b[:], AF.Exp,
                                         bias=nm_b[:], scale=1.0,
                                         accum_out=s_b[:])
                    nc.vector.reciprocal(rs_b[:], s_b[:])
                    nc.vector.tensor_scalar_mul(e_b[:], e_b[:], rs_b[:])
                    nc.sync.dma_start(ow[b:b + 1, :], e_b[:])

                    # ---- w transpose + ctx ----
                    pc0 = pC.tile([1, 512], f32, tag="pc0")
                    pc1 = pC.tile([1, 512], f32, tag="pc1")
                    for k in range(NTK):
                        kw = 128 if k < 12 else 64
                        trp = pW.tile([128, 1], f32)
                        nc.tensor.transpose(trp[0:kw, :],
                                            e_b[:, k * 128:k * 128 + kw],
                                            ident_sb[0:1, 0:1])
                        wtk = pwT.tile([128, 1], bf16)
                        nc.vector.tensor_copy(wtk[0:kw, :], trp[0:kw, :])
                        rhs = en[:, k, :] if k < 12 else ent[:]
                        nc.tensor.matmul(pc0[:], wtk[0:kw, :],
                                         rhs[0:kw, 0:512],
                                         start=(k == 0), stop=(k == NTK - 1))
                        nc.tensor.matmul(pc1[:], wtk[0:kw, :],
                                         rhs[0:kw, 512:1024],
                                         start=(k == 0), stop=(k == NTK - 1))
                    ctx_b = psm.tile([1, E], f32, tag="ctx")
                    nc.scalar.activation(ctx_b[:, 0:512], pc0[:], AF.Copy)
                    nc.scalar.activation(ctx_b[:, 512:1024], pc1[:], AF.Copy)
                    nc.sync.dma_start(octx[b:b + 1, :], ctx_b[:])

    nc.compile()
    return nc


def _round_f32r(x):
    """Round fp32 to fp32r (11-bit mantissa, RNE) — what the PE consumes."""
    u = np.ascontiguousarray(x, np.float32).view(np.uint32)
    lsb = (u >> np.uint32(12)) & np.uint32(1)
    u = u + np.uint32(0x7FF) + lsb
    u &= np.uint32(0xFFFFF000)
    return u.view(np.float32)


def _prep_inputs(enc_hs_pad, enc_hs_len, dec_z, att_prev, att_h, att_c,
                 W_enc, b_enc, W_dec, conv_w, W_ih, W_hh, W_g, b_g):
    bf = ml_dtypes.bfloat16
    f = np.float32

    # encT swizzled: [b, half, p, k, t] with e = half*512 + k*128 + p
    encT32 = _round_f32r(np.ascontiguousarray(enc_hs_pad.transpose(0, 2, 1),
                                              dtype=f))
    encT_sw = np.ascontiguousarray(
        encT32.reshape(B, 2, 4, 128, T).transpose(0, 1, 3, 2, 4))

    # encN swizzled: [b, p, k, e] with t = k*128 + p  (+ tail rows)
    encN32 = np.asarray(enc_hs_pad, dtype=f)
    encN_sw = np.ascontiguousarray(
        encN32[:, 0:1536, :].reshape(B, 12, 128, E).transpose(0, 2, 1, 3)
    ).astype(bf)
    encN_tl = np.ascontiguousarray(encN32[:, 1536:T, :]).astype(bf)

    att_pad = np.zeros((B, T + 2 * FILT), dtype=f)
    att_pad[:, FILT:FILT + T] = att_prev
    xcv = np.ascontiguousarray(
        np.lib.stride_tricks.sliding_window_view(att_pad, T, axis=1))

    wencT = _round_f32r(np.ascontiguousarray(W_enc.T, dtype=f))       # (E, A)
    wencT_sw = np.ascontiguousarray(
        wencT.reshape(8, 128, A).transpose(1, 0, 2))                  # (128,8,A)
    wg = _round_f32r(np.ascontiguousarray(W_g[:, None], dtype=f))     # (A,1)
    wg_sw = np.ascontiguousarray(wg.reshape(4, 128, 1).transpose(1, 0, 2))
    cvT = _round_f32r(np.ascontiguousarray(conv_w[:, 0, :].T, dtype=f))
    wihT = _round_f32r(np.ascontiguousarray(W_ih.T, dtype=f))
    whhT = np.ascontiguousarray(W_hh.T, dtype=f)                      # (A, G4)
    whh_sw = np.ascontiguousarray(
        whhT.reshape(4, 128, 4, 512).transpose(2, 1, 0, 3)).astype(bf)
    wdT = _round_f32r(np.ascontiguousarray(W_dec.T, dtype=f))         # (D, A)
    wdT_sw = np.ascontiguousarray(wdT.reshape(8, 128, A).transpose(1, 0, 2))
    wdTt = _round_f32r(np.ascontiguousarray(b_enc[None, :], dtype=f))
    maskS = np.where(np.arange(T)[None, :] < np.asarray(enc_hs_len)[:, None],
                     np.float32(0.0), np.float32(NEG)).astype(f)
    ident = np.eye(128, dtype=f)

    dzT_full = _round_f32r(np.ascontiguousarray(np.asarray(dec_z, dtype=f).T))
    athT_full = np.ascontiguousarray(np.asarray(att_h, dtype=f).T).astype(bf)

    in_maps = []
    for c in range(NCORES):
        s = slice(c * NB, (c + 1) * NB)
        dzT_c = np.ascontiguousarray(dzT_full[:, s])                  # (D, NB)
        in_maps.append({
            "encT": np.ascontiguousarray(encT_sw[s]),
            "encN": np.ascontiguousarray(encN_sw[s]),
            "encNt": np.ascontiguousarray(encN_tl[s]),
            "wencT": wencT_sw,
            "wg": wg_sw,
            "xcv": _round_f32r(np.ascontiguousarray(xcv[s])),
            "cvT": cvT,
            "wihT": wihT,
            "whh": whh_sw,
            "athT": np.ascontiguousarray(
                athT_full[:, s].reshape(4, 128, NB).transpose(1, 0, 2)),
            "atc": np.ascontiguousarray(np.asarray(att_c, dtype=f)[s]),
            "dzT": np.ascontiguousarray(
                dzT_c.reshape(8, 128, NB).transpose(1, 0, 2)),
            "dzTt": np.ones((1, NB), dtype=f),
            "wdT": wdT_sw,
            "wdTt": wdTt,
            "maskS": np.ascontiguousarray(maskS[s]),
            "ident": ident,
        })
    return in_maps


def kernel(**inputs):
    global _BUILT
    from concourse import bass_utils

    if _BUILT is None:
        _BUILT = _build()
    nc = _BUILT

    in_maps = _prep_inputs(**{k: np.asarray(v) for k, v in inputs.items()})
    res = bass_utils.run_bass_kernel_spmd(nc, in_maps,
                                          core_ids=list(range(NCORES)))
    ctx = np.concatenate([res.results[c]["octx"] for c in range(NCORES)], 0)
    w = np.concatenate([res.results[c]["ow"] for c in range(NCORES)], 0)
    h = np.concatenate([res.results[c]["oh"] for c in range(NCORES)], 0)
    cc = np.concatenate([res.results[c]["oc"] for c in range(NCORES)], 0)
    return ctx, w, h, cc
